# revision 17
# baseline (speedup 1.0000x reference)
"""MoE (top-2 of 8 experts) Trainium2 kernel, expert-parallel across 8 NeuronCores.

Strategy (matches the expert-parallel sharding hint):
  * Router runs on device, data-parallel over tokens (each core computes gate
    logits + top-2 + softmax weights for its 1/8 of the tokens).
  * Host performs only the integer dispatch planning (argwhere/pad) — the
    "all-to-all by top-k assignment" data movement of the full-IO contract.
  * Each core holds one expert's W1/W2 and runs the dense FFN over the tokens
    routed to it (float32r matmuls at full PE rate), applies the gate weight
    on device, and returns its compact expert output.
  * Host scatter-adds the 8 compact outputs back to token order (unshard).
"""

import numpy as np

import concourse.bacc as bacc
import concourse.mybir as mybir
from concourse.tile import TileContext
from concourse import bass_utils

F32 = mybir.dt.float32
F32R = mybir.dt.float32r
U32 = mybir.dt.uint32
AF = mybir.ActivationFunctionType
OP = mybir.AluOpType

B, T, D, F, E, KTOP = 4, 2048, 1024, 2048, 8, 2
N_CORES = 8
TOK = B * T                  # 8192 tokens
TPC = TOK // N_CORES         # 1024 router tokens per core
NT = TPC // 128              # 8 token tiles per core (router)
ND = D // 128                # 8 d-chunks
NF = F // 128                # 16 f-tiles
CHUNK = 384                  # moving-dim chunk (>=256 keeps f32r at full rate)
GROUP = 3 * CHUNK            # 1152 tokens per FFN group

_cache: dict = {}


def _new_nc():
    return bacc.Bacc(
        "TRN2", target_bir_lowering=False, debug=False, num_devices=N_CORES
    )


def build_router():
    """Per core: logits = x[c] @ gate_w + gate_b -> top2 -> softmax weights.

    Inputs : xt [D, TPC] (x shard transposed), gw [D, E], gb [128, E] (rows
             replicated).
    Outputs: ow [128, NT*2] f32 (w1, w2 per token tile), oi [128, NT*2] u32
             (expert ids).  Token (tile j, partition p) -> cols 2j, 2j+1.
    """
    nc = _new_nc()
    xt = nc.dram_tensor("xt", [D, TPC], F32, kind="ExternalInput")
    gw = nc.dram_tensor("gw", [128, ND * E], F32, kind="ExternalInput")
    gb = nc.dram_tensor("gb", [E, 1], F32, kind="ExternalInput")
    i8 = nc.dram_tensor("i8", [E, E], F32, kind="ExternalInput")
    ow = nc.dram_tensor("ow", [128, NT * 2], F32, kind="ExternalOutput")
    oi = nc.dram_tensor("oi", [128, NT * 2], U32, kind="ExternalOutput")

    with TileContext(nc) as tc:
        with (
            tc.tile_pool(name="xts", bufs=1) as xt_pool,
            tc.tile_pool(name="cst", bufs=1) as cst_pool,
            tc.tile_pool(name="wrk", bufs=4) as wrk_pool,
            tc.tile_pool(name="out", bufs=1) as out_pool,
            tc.tile_pool(name="ps", bufs=2, space="PSUM") as psum_pool,
            tc.tile_pool(name="pst", bufs=4, space="PSUM") as pst_pool,
        ):
            gw_sb = cst_pool.tile([128, ND * E], F32)
            nc.gpsimd.dma_start(out=gw_sb[:], in_=gw[:])
            gb_sb = cst_pool.tile([E, 1], F32)
            nc.gpsimd.dma_start(out=gb_sb[:], in_=gb[:])
            id8 = cst_pool.tile([E, E], F32)
            nc.gpsimd.dma_start(out=id8[:], in_=i8[:])

            xt_sb = []
            for d in range(ND):
                t = xt_pool.tile([128, TPC], F32, tag=f"xt{d}")
                nc.sync.dma_start(out=t[:], in_=xt[d * 128:(d + 1) * 128, :])
                xt_sb.append(t)

            # logitsT[e, t] accumulated over d-chunks, gate weights stationary
            NCH = TPC // 512
            lt_ps = [
                psum_pool.tile([E, 512], F32, name=f"ltps{ch}")
                for ch in range(NCH)
            ]
            for d in range(ND):
                for ch in range(NCH):
                    nc.tensor.matmul(
                        lt_ps[ch][:],
                        lhsT=gw_sb[:, d * E:(d + 1) * E],
                        rhs=xt_sb[d][:, ch * 512:(ch + 1) * 512],
                        start=(d == 0),
                        stop=(d == ND - 1),
                    )
            ltT = wrk_pool.tile([E, TPC], F32, tag="ltT")
            for ch in range(NCH):
                nc.vector.tensor_tensor(
                    ltT[:, ch * 512:(ch + 1) * 512],
                    lt_ps[ch][:],
                    gb_sb[:, 0:1].to_broadcast([E, 512]),
                    op=OP.add,
                )

            ow_sb = out_pool.tile([128, NT * 2], F32)
            oi_sb = out_pool.tile([128, NT * 2], U32)

            for j in range(NT):
                pst = pst_pool.tile([128, E], F32, tag="pst")
                nc.tensor.transpose(
                    out=pst[:],
                    in_=ltT[:, j * 128:(j + 1) * 128],
                    identity=id8[:],
                )
                logit = wrk_pool.tile([128, E], F32, tag="logit")
                nc.vector.tensor_copy(logit[:], pst[:])
                maxv = wrk_pool.tile([128, 8], F32, tag="maxv")
                maxi = wrk_pool.tile([128, 8], U32, tag="maxi")
                nc.vector.max_with_indices(maxv[:], maxi[:], logit[:])
                diff = wrk_pool.tile([128, 1], F32, tag="diff")
                nc.vector.tensor_sub(diff[:], maxv[:, 0:1], maxv[:, 1:2])
                # softmax over the two selected logits == sigmoid(+-diff)
                nc.scalar.activation(
                    ow_sb[:, 2 * j:2 * j + 1], diff[:], AF.Sigmoid
                )
                nc.scalar.activation(
                    ow_sb[:, 2 * j + 1:2 * j + 2], diff[:], AF.Sigmoid,
                    scale=-1.0,
                )
                nc.vector.tensor_copy(oi_sb[:, 2 * j:2 * j + 2], maxi[:, 0:2])

            nc.sync.dma_start(out=ow[:], in_=ow_sb[:])
            nc.sync.dma_start(out=oi[:], in_=oi_sb[:])
    nc.compile()
    return nc


def build_ffn(C: int):
    """Per core: yT = ((gelu(xg @ W1 + b1) @ W2) + b2) * gate, transposed IO.

    Inputs : xgt [D, C] gathered tokens (transposed), w1p [128, NF*1024]
             (packed: [p, ft*1024 + d*128 + c] = w1[d*128+p, ft*128+c]),
             w2p [128, ND*2048] ([p, dt*2048 + ft*128 + c] = w2[ft*128+p,
             dt*128+c]), b1s [128, NF], b2s [128, ND], gat [128, C].
    Output : yt [D, C].
    """
    assert C % GROUP == 0
    NG = C // GROUP
    nc = _new_nc()
    xgt = nc.dram_tensor("xgt", [D, C], F32R, kind="ExternalInput")
    w1p = nc.dram_tensor("w1p", [128, NF * ND * 128], F32R, kind="ExternalInput")
    w2p = nc.dram_tensor("w2p", [128, ND * NF * 128], F32R, kind="ExternalInput")
    b1s = nc.dram_tensor("b1s", [128, NF], F32, kind="ExternalInput")
    b2s = nc.dram_tensor("b2s", [128, ND], F32, kind="ExternalInput")
    gat = nc.dram_tensor("gat", [128, C], F32, kind="ExternalInput")
    yt = nc.dram_tensor("yt", [D, C], F32, kind="ExternalOutput")

    with TileContext(nc) as tc:
        with (
            tc.tile_pool(name="cst", bufs=1) as cst_pool,
            tc.tile_pool(name="xg", bufs=30) as xg_pool,
            tc.tile_pool(name="ht", bufs=NF) as ht_pool,
            tc.tile_pool(name="w1p", bufs=8) as w1_pool,
            tc.tile_pool(name="w2p", bufs=4) as w2_pool,
            tc.tile_pool(name="yo", bufs=4) as y_pool,
            tc.tile_pool(name="ps1", bufs=5, space="PSUM") as ps1_pool,
            tc.tile_pool(name="ps2", bufs=3, space="PSUM") as ps2_pool,
        ):
            b1_sb = cst_pool.tile([128, NF], F32)
            nc.gpsimd.dma_start(out=b1_sb[:], in_=b1s[:])
            b2_sb = cst_pool.tile([128, ND], F32)
            nc.gpsimd.dma_start(out=b2_sb[:], in_=b2s[:])
            gat_sb = cst_pool.tile([128, C], F32)

            # PE warmup: get HAM to full clock while first DMAs land
            warm = cst_pool.tile([128, 256], F32)
            nc.vector.memset(warm[:], 0.0)
            warm_ps = ps2_pool.tile([128, 256], F32, tag="ps2", name="warm_ps")
            for _ in range(5):
                nc.tensor.matmul(warm_ps[:], lhsT=warm[:, 0:128],
                                 rhs=warm[:, 0:256], start=True, stop=True)

            for g in range(NG):
                g0 = g * GROUP
                # gathered tokens, chunk-granular so mm1 starts early
                xg_sb = [[None] * 3 for _ in range(ND)]
                ring = {0: nc.sync, 1: nc.gpsimd, 2: nc.gpsimd}
                for ch in range(3):
                    for d in range(ND):
                        t = xg_pool.tile([128, CHUNK], F32R, tag="xg",
                                         name=f"xg_{g}_{d}_{ch}")
                        ring[ch].dma_start(
                            out=t[:],
                            in_=xgt[d * 128:(d + 1) * 128,
                                    g0 + ch * CHUNK:g0 + (ch + 1) * CHUNK],
                        )
                        xg_sb[d][ch] = t
                if g == 0:
                    nc.gpsimd.dma_start(out=gat_sb[:], in_=gat[:])
                ht_sb = []
                for ft in range(NF):
                    w1t = w1_pool.tile([128, ND * 128], F32R, tag="w1t",
                                       name=f"w1t_{g}_{ft}")
                    nc.scalar.dma_start(
                        out=w1t[:],
                        in_=w1p[:, ft * ND * 128:(ft + 1) * ND * 128],
                    )
                    ps = [
                        ps1_pool.tile([128, CHUNK], F32, tag="ps1",
                                      name=f"ps1_{g}_{ft}_{ch}")
                        for ch in range(3)
                    ]
                    if g == 0 and ft == 0:
                        # ch-outer: start on chunk 0 before ch1/ch2 DMAs land
                        for ch in range(3):
                            for d in range(ND):
                                nc.tensor.matmul(
                                    ps[ch][:],
                                    lhsT=w1t[:, d * 128:(d + 1) * 128],
                                    rhs=xg_sb[d][ch][:],
                                    start=(d == 0),
                                    stop=(d == ND - 1),
                                )
                    else:
                        for d in range(ND):
                            for ch in range(3):
                                nc.tensor.matmul(
                                    ps[ch][:],
                                    lhsT=w1t[:, d * 128:(d + 1) * 128],
                                    rhs=xg_sb[d][ch][:],
                                    start=(d == 0),
                                    stop=(d == ND - 1),
                                )
                    ht = ht_pool.tile([128, GROUP], F32R, tag="ht")
                    for ch in range(3):
                        nc.scalar.activation(
                            ht[:, ch * CHUNK:(ch + 1) * CHUNK],
                            ps[ch][:],
                            AF.Gelu,
                            bias=b1_sb[:, ft:ft + 1],
                        )
                    ht_sb.append(ht)
                for dt in range(ND):
                    w2t = w2_pool.tile([128, NF * 128], F32R, tag="w2t",
                                       name=f"w2t_{g}_{dt}")
                    nc.scalar.dma_start(
                        out=w2t[:],
                        in_=w2p[:, dt * NF * 128:(dt + 1) * NF * 128],
                    )
                    ps = [
                        ps2_pool.tile([128, CHUNK], F32, tag="ps2",
                                      name=f"ps2_{g}_{dt}_{ch}")
                        for ch in range(3)
                    ]
                    for ft in range(NF):
                        for ch in range(3):
                            nc.tensor.matmul(
                                ps[ch][:],
                                lhsT=w2t[:, ft * 128:(ft + 1) * 128],
                                rhs=ht_sb[ft][:, ch * CHUNK:(ch + 1) * CHUNK],
                                start=(ft == 0),
                                stop=(ft == NF - 1),
                            )
                    for ch in range(3):
                        ys = y_pool.tile([128, CHUNK], F32, tag="ys")
                        nc.vector.tensor_tensor(
                            ys[:],
                            ps[ch][:],
                            b2_sb[:, dt:dt + 1].to_broadcast([128, CHUNK]),
                            op=OP.add,
                        )
                        nc.vector.tensor_tensor(
                            ys[:],
                            ys[:],
                            gat_sb[:, g0 + ch * CHUNK:g0 + (ch + 1) * CHUNK],
                            op=OP.mult,
                        )
                        st_ring = nc.sync if g == NG - 1 else nc.gpsimd
                        st_ring.dma_start(
                            out=yt[
                                dt * 128:(dt + 1) * 128,
                                g0 + ch * CHUNK:g0 + (ch + 1) * CHUNK,
                            ],
                            in_=ys[:],
                        )
    nc.compile()
    return nc


def _run(nc, in_maps):
    res = bass_utils.run_bass_kernel_spmd(
        nc, in_maps, core_ids=list(range(N_CORES))
    )
    return res.results


def kernel(x, gate_w, gate_b, w1, b1, w2, b2):
    x = np.asarray(x, dtype=np.float32)
    gate_w = np.asarray(gate_w, dtype=np.float32)
    gate_b = np.asarray(gate_b, dtype=np.float32)
    w1 = np.asarray(w1, dtype=np.float32)
    b1 = np.asarray(b1, dtype=np.float32)
    w2 = np.asarray(w2, dtype=np.float32)
    b2 = np.asarray(b2, dtype=np.float32)

    xf = np.ascontiguousarray(x.reshape(TOK, D))

    # ---- Kernel 1: router --------------------------------------------------
    if "router" not in _cache:
        _cache["router"] = build_router()
    gb_rep = np.ascontiguousarray(gate_b.reshape(E, 1))
    in_maps = [
        {
            "xt": np.ascontiguousarray(xf[c * TPC:(c + 1) * TPC, :].T),
            "gw": np.ascontiguousarray(
                gate_w.reshape(ND, 128, E).transpose(1, 0, 2).reshape(128, ND * E)
            ),
            "gb": gb_rep,
            "i8": np.eye(E, dtype=np.float32),
        }
        for c in range(N_CORES)
    ]
    r = _run(_cache["router"], in_maps)

    top_i = np.empty((TOK, 2), dtype=np.int64)
    top_w = np.empty((TOK, 2), dtype=np.float32)
    for c in range(N_CORES):
        ow = r[c]["ow"]          # [128, NT*2]
        oi = r[c]["oi"].astype(np.int64)
        for j in range(NT):
            t0 = c * TPC + j * 128
            top_w[t0:t0 + 128, 0] = ow[:, 2 * j]
            top_w[t0:t0 + 128, 1] = ow[:, 2 * j + 1]
            top_i[t0:t0 + 128, 0] = oi[:, 2 * j]
            top_i[t0:t0 + 128, 1] = oi[:, 2 * j + 1]

    # ---- Host dispatch planning (pure indexing) ---------------------------
    ids = []
    gates = []
    for e in range(E):
        sel = top_i == e                       # [TOK, 2]
        tok = np.nonzero(sel.any(axis=1))[0]
        k = sel[tok, 1].astype(np.int64)       # which of the two slots
        ids.append(tok)
        gates.append(top_w[tok, k])
    nmax = max(len(t) for t in ids)
    C = ((nmax + GROUP - 1) // GROUP) * GROUP

    key = ("ffn", C)
    if key not in _cache:
        _cache[key] = build_ffn(C)

    in_maps = []
    for c in range(N_CORES):
        n = len(ids[c])
        xg = np.zeros((C, D), dtype=np.float32)
        xg[:n] = xf[ids[c]]
        gat = np.zeros((C,), dtype=np.float32)
        gat[:n] = gates[c]
        # [p, ft, d, c] = w1[d*128+p, ft*128+c]
        w1pk = np.ascontiguousarray(
            w1[c].reshape(ND, 128, NF, 128).transpose(1, 2, 0, 3)
            .reshape(128, NF * ND * 128)
        )
        # [p, dt, ft, c] = w2[ft*128+p, dt*128+c]
        w2pk = np.ascontiguousarray(
            w2[c].reshape(NF, 128, ND, 128).transpose(1, 2, 0, 3)
            .reshape(128, ND * NF * 128)
        )
        in_maps.append(
            {
                "xgt": np.ascontiguousarray(xg.T),
                "w1p": w1pk,
                "w2p": w2pk,
                "b1s": np.ascontiguousarray(b1[c].reshape(NF, 128).T),
                "b2s": np.ascontiguousarray(b2[c].reshape(ND, 128).T),
                "gat": np.ascontiguousarray(
                    np.broadcast_to(gat[None, :], (128, C))
                ),
            }
        )
    rf = _run(_cache[key], in_maps)

    # ---- Unshard: scatter-add compact expert outputs ----------------------
    out = np.zeros((TOK, D), dtype=np.float32)
    for c in range(N_CORES):
        n = len(ids[c])
        out[ids[c]] += rf[c]["yt"].T[:n]
    return out.reshape(B, T, D)


# revision 19
# speedup vs baseline: 1.0462x; 1.0462x over previous
"""MoE (top-2 of 8 experts) Trainium2 kernel, expert-parallel across 8 NeuronCores.

Strategy (matches the expert-parallel sharding hint):
  * Router runs on device, data-parallel over tokens (each core computes gate
    logits + top-2 + softmax weights for its 1/8 of the tokens).
  * Host performs only the integer dispatch planning (argwhere/pad) — the
    "all-to-all by top-k assignment" data movement of the full-IO contract.
  * Each core holds one expert's W1/W2 and runs the dense FFN over the tokens
    routed to it (float32r matmuls at full PE rate), applies the gate weight
    on device, and returns its compact expert output.
  * Host scatter-adds the 8 compact outputs back to token order (unshard).
"""

import numpy as np

import concourse.bacc as bacc
import concourse.mybir as mybir
from concourse.tile import TileContext
from concourse import bass_utils

F32 = mybir.dt.float32
F32R = mybir.dt.float32r
U32 = mybir.dt.uint32
AF = mybir.ActivationFunctionType
OP = mybir.AluOpType

B, T, D, F, E, KTOP = 4, 2048, 1024, 2048, 8, 2
N_CORES = 8
TOK = B * T                  # 8192 tokens
TPC = TOK // N_CORES         # 1024 router tokens per core
NT = TPC // 128              # 8 token tiles per core (router)
ND = D // 128                # 8 d-chunks
NF = F // 128                # 16 f-tiles
CHUNK = 384                  # moving-dim chunk (>=256 keeps f32r at full rate)
GROUP = 3 * CHUNK            # 1152 tokens per FFN group

_cache: dict = {}


def _new_nc():
    return bacc.Bacc(
        "TRN2", target_bir_lowering=False, debug=False, num_devices=N_CORES
    )


def build_router():
    """Per core: logits = x[c] @ gate_w + gate_b -> top2 -> softmax weights.

    Inputs : xt [D, TPC] (x shard transposed), gw [D, E], gb [128, E] (rows
             replicated).
    Outputs: ow [128, NT*2] f32 (w1, w2 per token tile), oi [128, NT*2] u32
             (expert ids).  Token (tile j, partition p) -> cols 2j, 2j+1.
    """
    nc = _new_nc()
    xt = nc.dram_tensor("xt", [D, TPC], F32, kind="ExternalInput")
    gw = nc.dram_tensor("gw", [128, ND * E], F32, kind="ExternalInput")
    gb = nc.dram_tensor("gb", [E, 1], F32, kind="ExternalInput")
    i8 = nc.dram_tensor("i8", [E, E], F32, kind="ExternalInput")
    ow = nc.dram_tensor("ow", [128, NT * 2], F32, kind="ExternalOutput")
    oi = nc.dram_tensor("oi", [128, NT * 2], U32, kind="ExternalOutput")

    with TileContext(nc) as tc:
        with (
            tc.tile_pool(name="xts", bufs=1) as xt_pool,
            tc.tile_pool(name="cst", bufs=1) as cst_pool,
            tc.tile_pool(name="wrk", bufs=4) as wrk_pool,
            tc.tile_pool(name="out", bufs=1) as out_pool,
            tc.tile_pool(name="ps", bufs=2, space="PSUM") as psum_pool,
            tc.tile_pool(name="pst", bufs=4, space="PSUM") as pst_pool,
        ):
            gw_sb = cst_pool.tile([128, ND * E], F32)
            nc.gpsimd.dma_start(out=gw_sb[:], in_=gw[:])
            gb_sb = cst_pool.tile([E, 1], F32)
            nc.gpsimd.dma_start(out=gb_sb[:], in_=gb[:])
            id8 = cst_pool.tile([E, E], F32)
            nc.gpsimd.dma_start(out=id8[:], in_=i8[:])

            xt_sb = []
            for d in range(ND):
                t = xt_pool.tile([128, TPC], F32, tag=f"xt{d}")
                nc.sync.dma_start(out=t[:], in_=xt[d * 128:(d + 1) * 128, :])
                xt_sb.append(t)

            # logitsT[e, t] accumulated over d-chunks, gate weights stationary
            NCH = TPC // 512
            lt_ps = [
                psum_pool.tile([E, 512], F32, name=f"ltps{ch}")
                for ch in range(NCH)
            ]
            for d in range(ND):
                for ch in range(NCH):
                    nc.tensor.matmul(
                        lt_ps[ch][:],
                        lhsT=gw_sb[:, d * E:(d + 1) * E],
                        rhs=xt_sb[d][:, ch * 512:(ch + 1) * 512],
                        start=(d == 0),
                        stop=(d == ND - 1),
                    )
            ltT = wrk_pool.tile([E, TPC], F32, tag="ltT")
            for ch in range(NCH):
                nc.vector.tensor_tensor(
                    ltT[:, ch * 512:(ch + 1) * 512],
                    lt_ps[ch][:],
                    gb_sb[:, 0:1].to_broadcast([E, 512]),
                    op=OP.add,
                )

            ow_sb = out_pool.tile([128, NT * 2], F32)
            oi_sb = out_pool.tile([128, NT * 2], U32)

            for j in range(NT):
                pst = pst_pool.tile([128, E], F32, tag="pst")
                nc.tensor.transpose(
                    out=pst[:],
                    in_=ltT[:, j * 128:(j + 1) * 128],
                    identity=id8[:],
                )
                logit = wrk_pool.tile([128, E], F32, tag="logit")
                nc.vector.tensor_copy(logit[:], pst[:])
                maxv = wrk_pool.tile([128, 8], F32, tag="maxv")
                maxi = wrk_pool.tile([128, 8], U32, tag="maxi")
                nc.vector.max_with_indices(maxv[:], maxi[:], logit[:])
                diff = wrk_pool.tile([128, 1], F32, tag="diff")
                nc.vector.tensor_sub(diff[:], maxv[:, 0:1], maxv[:, 1:2])
                # softmax over the two selected logits == sigmoid(+-diff)
                nc.scalar.activation(
                    ow_sb[:, 2 * j:2 * j + 1], diff[:], AF.Sigmoid
                )
                nc.scalar.activation(
                    ow_sb[:, 2 * j + 1:2 * j + 2], diff[:], AF.Sigmoid,
                    scale=-1.0,
                )
                nc.vector.tensor_copy(oi_sb[:, 2 * j:2 * j + 2], maxi[:, 0:2])

            nc.sync.dma_start(out=ow[:], in_=ow_sb[:])
            nc.sync.dma_start(out=oi[:], in_=oi_sb[:])
    nc.compile()
    return nc


def build_ffn(C: int):
    """Per core: yT = ((gelu(xg @ W1 + b1) @ W2) + b2) * gate, transposed IO.

    Inputs : xgt [D, C] gathered tokens (transposed), w1p [128, NF*1024]
             (packed: [p, ft*1024 + d*128 + c] = w1[d*128+p, ft*128+c]),
             w2p [128, ND*2048] ([p, dt*2048 + ft*128 + c] = w2[ft*128+p,
             dt*128+c]), b1s [128, NF], b2s [128, ND], gat [128, C].
    Output : yt [D, C].
    """
    assert C % GROUP == 0
    NG = C // GROUP
    nc = _new_nc()
    xgt = nc.dram_tensor("xgt", [D, C], F32R, kind="ExternalInput")
    w1p = nc.dram_tensor("w1p", [128, NF * ND * 128], F32R, kind="ExternalInput")
    w2p = nc.dram_tensor("w2p", [128, ND * NF * 128], F32R, kind="ExternalInput")
    b1s = nc.dram_tensor("b1s", [128, NF], F32, kind="ExternalInput")
    b2s = nc.dram_tensor("b2s", [128, ND], F32, kind="ExternalInput")
    gat = nc.dram_tensor("gat", [128, C], F32, kind="ExternalInput")
    yt = nc.dram_tensor("yt", [D, C], F32, kind="ExternalOutput")

    with TileContext(nc) as tc:
        with (
            tc.tile_pool(name="cst", bufs=1) as cst_pool,
            tc.tile_pool(name="xg", bufs=30) as xg_pool,
            tc.tile_pool(name="ht", bufs=NF) as ht_pool,
            tc.tile_pool(name="w1p", bufs=8) as w1_pool,
            tc.tile_pool(name="w2p", bufs=3) as w2_pool,
            tc.tile_pool(name="yo", bufs=6) as y_pool,
            tc.tile_pool(name="ps1", bufs=4, space="PSUM") as ps1_pool,
            tc.tile_pool(name="ps2", bufs=4, space="PSUM") as ps2_pool,
        ):
            b1_sb = cst_pool.tile([128, NF], F32)
            nc.gpsimd.dma_start(out=b1_sb[:], in_=b1s[:])
            b2_sb = cst_pool.tile([128, ND], F32)
            nc.gpsimd.dma_start(out=b2_sb[:], in_=b2s[:])
            gat_sb = cst_pool.tile([128, C], F32)

            # PE warmup: get HAM to full clock while first DMAs land
            warm = cst_pool.tile([128, 256], F32)
            nc.vector.memset(warm[:], 0.0)
            warm_ps = ps2_pool.tile([128, 256], F32, tag="ps2", name="warm_ps")
            for _ in range(5):
                nc.tensor.matmul(warm_ps[:], lhsT=warm[:, 0:128],
                                 rhs=warm[:, 0:256], start=True, stop=True)

            for g in range(NG):
                g0 = g * GROUP
                # gathered tokens, chunk-granular so mm1 starts early
                xg_sb = [[None] * 3 for _ in range(ND)]
                ring = {0: nc.sync, 1: nc.scalar, 2: nc.gpsimd}
                for ch in range(3):
                    for d in range(ND):
                        t = xg_pool.tile([128, CHUNK], F32R, tag="xg",
                                         name=f"xg_{g}_{d}_{ch}")
                        ring[ch].dma_start(
                            out=t[:],
                            in_=xgt[d * 128:(d + 1) * 128,
                                    g0 + ch * CHUNK:g0 + (ch + 1) * CHUNK],
                        )
                        xg_sb[d][ch] = t
                if g == 0:
                    nc.gpsimd.dma_start(out=gat_sb[:], in_=gat[:])
                ht_sb = []
                for ft in range(NF):
                    w1t = w1_pool.tile([128, ND * 128], F32R, tag="w1t",
                                       name=f"w1t_{g}_{ft}")
                    nc.scalar.dma_start(
                        out=w1t[:],
                        in_=w1p[:, ft * ND * 128:(ft + 1) * ND * 128],
                    )
                    ps = [
                        ps1_pool.tile([128, CHUNK], F32, tag="ps1",
                                      name=f"ps1_{g}_{ft}_{ch}")
                        for ch in range(3)
                    ]
                    if g == 0 and ft == 0:
                        # ch-outer: start on chunk 0 before ch1/ch2 DMAs land
                        for ch in range(3):
                            for d in range(ND):
                                nc.tensor.matmul(
                                    ps[ch][:],
                                    lhsT=w1t[:, d * 128:(d + 1) * 128],
                                    rhs=xg_sb[d][ch][:],
                                    start=(d == 0),
                                    stop=(d == ND - 1),
                                )
                    else:
                        for d in range(ND):
                            for ch in range(3):
                                nc.tensor.matmul(
                                    ps[ch][:],
                                    lhsT=w1t[:, d * 128:(d + 1) * 128],
                                    rhs=xg_sb[d][ch][:],
                                    start=(d == 0),
                                    stop=(d == ND - 1),
                                )
                    ht = ht_pool.tile([128, GROUP], F32R, tag="ht")
                    for ch in range(3):
                        nc.scalar.activation(
                            ht[:, ch * CHUNK:(ch + 1) * CHUNK],
                            ps[ch][:],
                            AF.Gelu,
                            bias=b1_sb[:, ft:ft + 1],
                        )
                    ht_sb.append(ht)
                for dt in range(ND):
                    w2t = w2_pool.tile([128, NF * 128], F32R, tag="w2t",
                                       name=f"w2t_{g}_{dt}")
                    nc.scalar.dma_start(
                        out=w2t[:],
                        in_=w2p[:, dt * NF * 128:(dt + 1) * NF * 128],
                    )
                    ps = [
                        ps2_pool.tile([128, CHUNK], F32, tag="ps2",
                                      name=f"ps2_{g}_{dt}_{ch}")
                        for ch in range(3)
                    ]
                    for ft in range(NF):
                        for ch in range(3):
                            nc.tensor.matmul(
                                ps[ch][:],
                                lhsT=w2t[:, ft * 128:(ft + 1) * 128],
                                rhs=ht_sb[ft][:, ch * CHUNK:(ch + 1) * CHUNK],
                                start=(ft == 0),
                                stop=(ft == NF - 1),
                            )
                    for ch in range(3):
                        ys = y_pool.tile([128, CHUNK], F32, tag="ys")
                        nc.vector.tensor_tensor(
                            ys[:],
                            ps[ch][:],
                            b2_sb[:, dt:dt + 1].to_broadcast([128, CHUNK]),
                            op=OP.add,
                        )
                        nc.vector.tensor_tensor(
                            ys[:],
                            ys[:],
                            gat_sb[:, g0 + ch * CHUNK:g0 + (ch + 1) * CHUNK],
                            op=OP.mult,
                        )
                        st_ring = nc.sync if g == NG - 1 else nc.gpsimd
                        st_ring.dma_start(
                            out=yt[
                                dt * 128:(dt + 1) * 128,
                                g0 + ch * CHUNK:g0 + (ch + 1) * CHUNK,
                            ],
                            in_=ys[:],
                        )
    nc.compile()
    return nc


def _run(nc, in_maps):
    res = bass_utils.run_bass_kernel_spmd(
        nc, in_maps, core_ids=list(range(N_CORES))
    )
    return res.results


def kernel(x, gate_w, gate_b, w1, b1, w2, b2):
    x = np.asarray(x, dtype=np.float32)
    gate_w = np.asarray(gate_w, dtype=np.float32)
    gate_b = np.asarray(gate_b, dtype=np.float32)
    w1 = np.asarray(w1, dtype=np.float32)
    b1 = np.asarray(b1, dtype=np.float32)
    w2 = np.asarray(w2, dtype=np.float32)
    b2 = np.asarray(b2, dtype=np.float32)

    xf = np.ascontiguousarray(x.reshape(TOK, D))

    # ---- Kernel 1: router --------------------------------------------------
    if "router" not in _cache:
        _cache["router"] = build_router()
    gb_rep = np.ascontiguousarray(gate_b.reshape(E, 1))
    in_maps = [
        {
            "xt": np.ascontiguousarray(xf[c * TPC:(c + 1) * TPC, :].T),
            "gw": np.ascontiguousarray(
                gate_w.reshape(ND, 128, E).transpose(1, 0, 2).reshape(128, ND * E)
            ),
            "gb": gb_rep,
            "i8": np.eye(E, dtype=np.float32),
        }
        for c in range(N_CORES)
    ]
    r = _run(_cache["router"], in_maps)

    top_i = np.empty((TOK, 2), dtype=np.int64)
    top_w = np.empty((TOK, 2), dtype=np.float32)
    for c in range(N_CORES):
        ow = r[c]["ow"]          # [128, NT*2]
        oi = r[c]["oi"].astype(np.int64)
        for j in range(NT):
            t0 = c * TPC + j * 128
            top_w[t0:t0 + 128, 0] = ow[:, 2 * j]
            top_w[t0:t0 + 128, 1] = ow[:, 2 * j + 1]
            top_i[t0:t0 + 128, 0] = oi[:, 2 * j]
            top_i[t0:t0 + 128, 1] = oi[:, 2 * j + 1]

    # ---- Host dispatch planning (pure indexing) ---------------------------
    ids = []
    gates = []
    for e in range(E):
        sel = top_i == e                       # [TOK, 2]
        tok = np.nonzero(sel.any(axis=1))[0]
        k = sel[tok, 1].astype(np.int64)       # which of the two slots
        ids.append(tok)
        gates.append(top_w[tok, k])
    nmax = max(len(t) for t in ids)
    C = ((nmax + GROUP - 1) // GROUP) * GROUP

    key = ("ffn", C)
    if key not in _cache:
        _cache[key] = build_ffn(C)

    in_maps = []
    for c in range(N_CORES):
        n = len(ids[c])
        xg = np.zeros((C, D), dtype=np.float32)
        xg[:n] = xf[ids[c]]
        gat = np.zeros((C,), dtype=np.float32)
        gat[:n] = gates[c]
        # [p, ft, d, c] = w1[d*128+p, ft*128+c]
        w1pk = np.ascontiguousarray(
            w1[c].reshape(ND, 128, NF, 128).transpose(1, 2, 0, 3)
            .reshape(128, NF * ND * 128)
        )
        # [p, dt, ft, c] = w2[ft*128+p, dt*128+c]
        w2pk = np.ascontiguousarray(
            w2[c].reshape(NF, 128, ND, 128).transpose(1, 2, 0, 3)
            .reshape(128, ND * NF * 128)
        )
        in_maps.append(
            {
                "xgt": np.ascontiguousarray(xg.T),
                "w1p": w1pk,
                "w2p": w2pk,
                "b1s": np.ascontiguousarray(b1[c].reshape(NF, 128).T),
                "b2s": np.ascontiguousarray(b2[c].reshape(ND, 128).T),
                "gat": np.ascontiguousarray(
                    np.broadcast_to(gat[None, :], (128, C))
                ),
            }
        )
    rf = _run(_cache[key], in_maps)

    # ---- Unshard: scatter-add compact expert outputs ----------------------
    out = np.zeros((TOK, D), dtype=np.float32)
    for c in range(N_CORES):
        n = len(ids[c])
        out[ids[c]] += rf[c]["yt"].T[:n]
    return out.reshape(B, T, D)
